# revision 13
# baseline (speedup 1.0000x reference)
"""HAttentionNetwork Trainium2 kernel (v7).

Strategy (8 NeuronCores, data-parallel over bags):
- 4096 bags are LPT-packed into 80 chunks (10/core, <=64 bags each),
  minimizing max sentences/chunk; sentences are gathered host-side so each
  chunk's sentences are contiguous and padded to Tc tiles of 128.
- DMA batching: per tile, the transposed x block (for logits) and the
  row-major x block (+ones) are packed side by side into one 514-col fp16
  record; 7 tiles' records ship per dma_start (~920 KB each), because each
  dma_start costs ~625ns of serialized HWDGE descriptor-generation and
  small transfers also waste SDMA efficiency. Per-sentence scalars
  (label, seg-rel; fp32 as is_equal requires) and all constants are
  resident, loaded by 3 one-time DMAs.
- Per 128-sentence tile, on device (row-major logits):
    FL[s, 112] = xt-block (two 128-col fp16 stationary, FWL) @ ct
                 (PE, PSUM f32; cols = [L0-by-class 53|pad | L1 53|pad];
                 both levels indexed by class so one label onehot serves both)
    E = exp(FL)                                   (ACT, -> fp16 SBUF)
    et[s, l] = sum_c (iota==lab) * E-block_l      (DVE scalar_tensor_tensor
                                                   with fused accum_out, x2)
    a2[s, l*64+b] = (iota64==seg_rel) * et_l      (DVE tensor_scalar x2, fp16)
    u2 += a2^T @ [x | 1]                          (PE, segment-sum, PSUM f32)
- Per chunk epilogue: 1/s scale (ACT), PE transpose, disc proj, +bias.
- Issue order is software-pipelined one stage: FL(t+1) goes to the PE queue
  before u2(t), so PE never stalls on the ACT/DVE chain of tile t.
Numerics: fp16 operands (11-bit mantissa; |logit|<8 so exp<3e3 fits),
fp32 PSUM accumulation everywhere.
DMA/tile: 128.5KB x-records + 1KB amortized scalars (~1036 B/sentence vs
1754 for the f32 baseline), in ~0.9MB transfers.
"""

import numpy as np

N_SENT = 262144
N_BAGS = 4096
HIDDEN = 256
L0 = 14
NCLS = 53
NCORE = 8
CHUNKS_PER_CORE = 10
NCHUNK = NCORE * CHUNKS_PER_CORE
MAX_BAGS_PER_CHUNK = 64
CW = 112  # class cols: [0:53]=L0-by-class (pad 56), [56:109]=L1-by-class (pad 112)
REC = 514  # per-tile record cols: [xt 256 | x 256 | ones 1 | pad 1]
QS = 7  # tiles per dma_start

_CACHE = {}


def _patch_tile_drain():
    # This walrus build rejects Drain instructions carrying more than ~1 sync
    # wait. Split the Tile final-drain waits across SP nops, one wait each.
    import concourse.mybir as mybir
    import concourse.tile as tile_mod
    from concourse.vector_clock import ScopedClock

    if getattr(tile_mod.TileContext, "_drain_split_patched", False):
        return

    def _split_drain_and_barrier(self, tick_clock, wait_clock):
        drain_inst = self.nc.sync.drain()
        wait_clock.add_sem_waits(
            drain_inst.ins, ScopedClock({None: tick_clock.global_clock})
        )
        si = drain_inst.ins.sync_info
        waits = list(si.on_wait) if si is not None else []
        if len(waits) > 1:
            drain_inst.ins.sync_info = mybir.SyncInfo(
                on_wait=waits[:1], on_update=list(si.on_update)
            )
            for w in waits[1:]:
                nop = self.nc.sync.nop(nofuse=True, hint="drain_wait_split")
                nop.ins.sync_info = mybir.SyncInfo(on_wait=[w], on_update=[])
        self.nc.all_engine_barrier()
        assert self.sems is not None
        popped = self.nc._tile_sem_poison_stack.pop()
        assert popped is self._sem_poison
        self.nc.clear_and_free_semaphores(list(self.sems.allocated().values()))
        self.nc.all_engine_barrier()

    tile_mod.TileContext._drain_and_barrier = _split_drain_and_barrier
    tile_mod.TileContext._drain_split_patched = True


def _split_all_waits(nc, max_waits=1):
    """This walrus build caps sync-wait commands per instruction very low.
    Move excess waits onto same-engine NOPs inserted just before."""
    import concourse.mybir as mybir

    n = 0
    for f in nc.m.functions:
        for b in f.blocks:
            new = []
            for inst in b.instructions:
                si = getattr(inst, "sync_info", None)
                waits = list(si.on_wait) if si is not None else []
                if len(waits) > max_waits:
                    keep = waits[:max_waits]
                    extra = waits[max_waits:]
                    for w in extra:
                        nop = mybir.InstNoOp(
                            name=f"waitsplit-{n}", ins=[], outs=[]
                        )
                        n += 1
                        nop.engine = inst.engine
                        nop.sync_info = mybir.SyncInfo(
                            on_wait=[w], on_update=[]
                        )
                        new.append(nop)
                    inst.sync_info = mybir.SyncInfo(
                        on_wait=keep, on_update=list(si.on_update)
                    )
                new.append(inst)
            b.instructions[:] = new
    return n


def _segment_ids(scope):
    marks = np.zeros(N_SENT, np.int64)
    np.add.at(marks, scope[1:-1].astype(np.int64), 1)
    return np.cumsum(marks)


def _lpt_chunks(counts):
    """LPT-pack all bags into NCHUNK chunks (<= MAX_BAGS_PER_CHUNK bags
    each), minimizing max sentence count. Returns list of bag-id lists."""
    order = np.argsort(-counts, kind="stable")
    loads = np.zeros(NCHUNK, np.int64)
    nbags = np.zeros(NCHUNK, np.int64)
    assign = [[] for _ in range(NCHUNK)]
    big = np.int64(1 << 60)
    for b in order:
        masked = np.where(nbags < MAX_BAGS_PER_CHUNK, loads, big)
        k = int(np.argmin(masked))
        assign[k].append(int(b))
        loads[k] += int(counts[b])
        nbags[k] += 1
    return assign, int(loads.max())


def _build_bass(Tc):
    import concourse.mybir as mybir
    from concourse import bass
    from concourse.tile import TileContext

    _patch_tile_drain()
    f32 = mybir.dt.float32
    f16 = mybir.dt.float16
    AO = mybir.AluOpType
    G = CHUNKS_PER_CORE * Tc
    NQ = (Tc + QS - 1) // QS

    nc = bass.Bass("TRN2")
    d_mg = nc.dram_tensor("mg3", [CHUNKS_PER_CORE, 128, Tc * REC], f16,
                          kind="ExternalInput")
    # f16 consts: [ct0 112 | ct1 112 | io56 56 | io64 64 | dt4 212] = 556
    d_cst = nc.dram_tensor("cst", [128, 556], f16, kind="ExternalInput")
    # f32 consts: [ident 128 | per-tile (label, seg-rel) 2G]
    d_cf = nc.dram_tensor("cf", [128, 128 + 2 * G], f32, kind="ExternalInput")
    d_bb = nc.dram_tensor("biasb", [64, 53], f32, kind="ExternalInput")
    d_out = nc.dram_tensor(
        "out", [CHUNKS_PER_CORE, 64, 53], f32, kind="ExternalOutput"
    )

    with TileContext(nc) as tc:
        with (
            tc.tile_pool(name="const", bufs=1) as cpool,
            tc.tile_pool(name="mgp", bufs=2 * NQ + 6) as mgp,
            tc.tile_pool(name="ep", bufs=3) as epool,
            tc.tile_pool(name="jp", bufs=3) as jpool,
            tc.tile_pool(name="etp", bufs=3) as etpool,
            tc.tile_pool(name="a2p", bufs=3) as a2pool,
            tc.tile_pool(name="miscp", bufs=2) as miscp,
            tc.tile_pool(name="ps_fl", bufs=2, space="PSUM") as ps_fl,
            tc.tile_pool(name="ps_u", bufs=2, space="PSUM") as ps_u,
            tc.tile_pool(name="ps_tp", bufs=1, space="PSUM") as ps_tp,
            tc.tile_pool(name="ps_o", bufs=1, space="PSUM") as ps_o,
        ):
            cst = cpool.tile([128, 556], f16, tag="cst")
            cf = cpool.tile([128, 128 + 2 * G], f32, tag="cf")
            bb = cpool.tile([64, 53], f32, tag="bb")
            nc.sync.dma_start(out=cst[:], in_=d_cst[:])
            nc.sync.dma_start(out=cf[:], in_=d_cf[:])
            nc.sync.dma_start(out=bb[:], in_=d_bb[:])
            ct0 = cst[:, 0:112]
            ct1 = cst[:, 112:224]
            io56 = cst[:, 224:280]
            io64 = cst[:, 280:344]
            identb = cf[:, 0:128]

            EXP = mybir.ActivationFunctionType.Exp
            CPY = mybir.ActivationFunctionType.Copy

            # one-stage software pipeline: tail(t-1) = [u2 matmul (+chunk
            # epilogue)] is issued after FL(t) so PE stays busy while the
            # ACT/DVE chain of tile t-1 produces a2(t-1).
            pending = [None]

            def epilogue(k, u2):
                seps = miscp.tile([128, 1], f32, tag="seps")
                invs = miscp.tile([128, 1], f32, tag="invs")
                nc.scalar.activation(
                    seps[:], u2[:, 256:257], CPY, bias=1e-30
                )
                nc.vector.reciprocal(invs[:], seps[:])
                repre = miscp.tile([128, 256], f32, tag="repre")
                nc.scalar.activation(
                    repre[:], u2[:, 0:256], CPY, scale=invs[:]
                )
                tp = ps_tp.tile([128, 256], f32, tag="tp")
                nc.tensor.transpose(tp[:, 0:128], repre[:, 0:128], identb)
                nc.tensor.transpose(tp[:, 128:256], repre[:, 128:256], identb)
                rT = miscp.tile([128, 256], f16, tag="rT")
                nc.scalar.copy(rT[:], tp[:])
                outp = ps_o.tile([64, 53], f32, tag="outp")
                for hh in range(2):
                    for l in range(2):
                        nc.tensor.matmul(
                            outp[:],
                            rT[:, hh * 128 + l * 64 : hh * 128 + l * 64 + 64],
                            cst[:, 344 + (hh * 2 + l) * 53 :
                                344 + (hh * 2 + l + 1) * 53],
                            start=(hh == 0 and l == 0),
                            stop=(hh == 1 and l == 1),
                        )
                outs = miscp.tile([64, 53], f32, tag="outs")
                nc.vector.tensor_tensor(outs[:], outp[:], bb[:], AO.add)
                nc.sync.dma_start(out=d_out[k], in_=outs[:])

            def flush_tail():
                if pending[0] is not None:
                    k, t, u2, a2, xh = pending[0]
                    nc.tensor.matmul(
                        u2[:], a2[:], xh,
                        start=(t == 0), stop=(t == Tc - 1),
                    )
                    if t == Tc - 1:
                        epilogue(k, u2)
                    pending[0] = None

            def slice_plan(k):
                # small leading slices on the first chunk cut pipeline fill;
                # small trailing slices on the last chunk cut the drain tail
                ws = []
                rem = Tc
                if k == 0:
                    for w in (2, 2, 3, 5):
                        if rem <= QS:
                            break
                        ws.append(w)
                        rem -= w
                tail = []
                if k == CHUNKS_PER_CORE - 1:
                    for w in (3, 2, 2):
                        if rem > QS:
                            tail.insert(0, w)
                            rem -= w
                while rem > 0:
                    w = min(QS, rem)
                    ws.append(w)
                    rem -= w
                return ws + tail

            u2 = None
            for k in range(CHUNKS_PER_CORE):
                # batched loads: up to QS tiles' records per dma_start
                plan = slice_plan(k)
                mgq = []  # per tile: (tile obj, local tile idx)
                t0 = 0
                for w in plan:
                    mt = mgp.tile([128, QS * REC], f16, tag="mg")
                    nc.sync.dma_start(
                        out=mt[:, 0 : w * REC],
                        in_=d_mg[k, :, t0 * REC : (t0 + w) * REC],
                    )
                    for j in range(w):
                        mgq.append((mt, j))
                    t0 += w
                for t in range(Tc):
                    g = k * Tc + t
                    mt, tl = mgq[t]
                    base = tl * REC
                    lab = cf[:, 128 + 2 * g : 129 + 2 * g]
                    sgr = cf[:, 129 + 2 * g : 130 + 2 * g]

                    if t == 0:
                        u2 = ps_u.tile([128, 257], f32, tag="u2")
                    fl = ps_fl.tile([128, CW], f32, tag="fl")
                    nc.tensor.matmul(
                        fl[:], mt[:, base : base + 128], ct0,
                        start=True, stop=False,
                    )
                    nc.tensor.matmul(
                        fl[:], mt[:, base + 128 : base + 256], ct1,
                        start=False, stop=True,
                    )
                    flush_tail()

                    E = epool.tile([128, CW], f16, tag="E")
                    nc.scalar.activation(E[:], fl[:], EXP)
                    et = etpool.tile([128, 2], f32, tag="et")
                    sj = jpool.tile([128, CW], f16, tag="sj")
                    # both class blocks share the label onehot (L0 gathered
                    # by class on host)
                    nc.vector.scalar_tensor_tensor(
                        sj[:, 0:56], io56, lab, E[:, 0:56],
                        AO.is_equal, AO.mult, accum_out=et[:, 0:1],
                    )
                    nc.vector.scalar_tensor_tensor(
                        sj[:, 56:112], io56, lab, E[:, 56:112],
                        AO.is_equal, AO.mult, accum_out=et[:, 1:2],
                    )
                    a2 = a2pool.tile([128, 128], f16, tag="a2")
                    nc.vector.tensor_scalar(
                        a2[:, 0:64], io64, sgr, et[:, 0:1],
                        AO.is_equal, AO.mult,
                    )
                    nc.vector.tensor_scalar(
                        a2[:, 64:128], io64, sgr, et[:, 1:2],
                        AO.is_equal, AO.mult,
                    )
                    pending[0] = (k, t, u2, a2,
                                  mt[:, base + 256 : base + 513])
            flush_tail()

    _split_all_waits(nc)
    return nc


def _prep(x, rel_emb0, rel_emb1, disc, bias, relation_levels, label_index, scope):
    seg = _segment_ids(np.asarray(scope))
    counts = np.bincount(seg, minlength=N_BAGS).astype(np.int64)
    cum = np.concatenate([[0], np.cumsum(counts)])
    assign, max_load = _lpt_chunks(counts)
    Tc = max(1, (max_load + 127) // 128)
    G = CHUNKS_PER_CORE * Tc
    f16 = np.float16

    x = np.asarray(x, np.float32)
    rl = np.asarray(relation_levels, np.int64)
    labels = np.asarray(label_index, np.float32)

    ctT = np.zeros((256, CW), np.float32)
    ctT[:, 0:53] = np.asarray(rel_emb0, np.float32)[rl[:, 0]].T   # L0 by class
    ctT[:, 56:109] = np.asarray(rel_emb1, np.float32)[rl[:, 1]].T  # L1 by class

    io56 = np.full((56,), 1000.0, np.float32)
    io56[0:53] = np.arange(53)
    io64v = np.arange(64, dtype=np.float32)
    disc = np.asarray(disc, np.float32)
    dt4 = np.zeros((128, 4, 53), np.float32)
    for hh in range(2):
        for l in range(2):
            dt4[:, hh * 2 + l, :] = disc[:, l * 256 + hh * 128 :
                                         l * 256 + (hh + 1) * 128].T
    cst = np.zeros((128, 556), np.float32)
    cst[:, 0:112] = ctT[0:128]
    cst[:, 112:224] = ctT[128:256]
    cst[:, 224:280] = io56[None, :]
    cst[:, 280:344] = io64v[None, :]
    cst[:, 344:556] = dt4.reshape(128, 212)
    cst = cst.astype(f16)

    biasb = np.broadcast_to(
        np.asarray(bias, np.float32), (64, 53)
    ).copy()

    in_maps = []
    meta = []
    for core in range(NCORE):
        mg3 = np.zeros((CHUNKS_PER_CORE, 128, Tc * REC), f16)
        cf = np.zeros((128, 128 + 2 * G), np.float32)
        cf[:, 0:128] = np.eye(128, dtype=np.float32)
        cmeta = []
        for kk in range(CHUNKS_PER_CORE):
            bags = assign[core * CHUNKS_PER_CORE + kk]
            if bags:
                sents = np.concatenate(
                    [np.arange(cum[b], cum[b + 1]) for b in bags]
                )
                srel = np.repeat(
                    np.arange(len(bags), dtype=np.float32),
                    counts[np.asarray(bags)],
                )
            else:
                sents = np.zeros((0,), np.int64)
                srel = np.zeros((0,), np.float32)
            L = len(sents)
            Xc = np.zeros((Tc * 128, 256), np.float32)
            Xc[0:L] = x[sents]
            rec = np.zeros((Tc, 128, REC), f16)
            # xt block: rec[t, p, blk*128 + s] = x-slot[t*128+s, blk*128+p]
            rec[:, :, 0:256] = (
                Xc.reshape(Tc, 128, 2, 128)
                .transpose(0, 3, 2, 1)
                .reshape(Tc, 128, 256)
                .astype(f16)
            )
            rec[:, :, 256:512] = Xc.astype(f16).reshape(Tc, 128, 256)
            col = np.zeros((Tc * 128,), np.float32)
            col[0:L] = 1.0
            rec[:, :, 512] = col.astype(f16).reshape(Tc, 128)
            mg3[kk] = rec.transpose(1, 0, 2).reshape(128, Tc * REC)
            lab = np.zeros((Tc * 128,), np.float32)
            lab[0:L] = labels[sents]
            sgr = np.full((Tc * 128,), -1.0, np.float32)
            sgr[0:L] = srel
            pair = np.stack(
                [lab.reshape(Tc, 128), sgr.reshape(Tc, 128)], axis=2
            )  # [Tc, 128, 2]
            cf[:, 128 + 2 * kk * Tc : 128 + 2 * (kk + 1) * Tc] = (
                pair.transpose(1, 0, 2).reshape(128, 2 * Tc)
            )
            cmeta.append(bags)
        meta.append(cmeta)
        in_maps.append({"mg3": mg3, "cf": cf, "cst": cst, "biasb": biasb})
    return Tc, in_maps, meta


def kernel(x, rel_emb0, rel_emb1, disc, bias, relation_levels, label_index,
           scope, _trace=False):
    from concourse.bass_utils import run_bass_kernel_spmd

    Tc, in_maps, meta = _prep(
        x, rel_emb0, rel_emb1, disc, bias, relation_levels, label_index, scope
    )
    if Tc not in _CACHE:
        _CACHE[Tc] = _build_bass(Tc)
    nc = _CACHE[Tc]
    res = None
    for attempt in range(3):
        try:
            res = run_bass_kernel_spmd(
                nc, in_maps, core_ids=list(range(NCORE)), trace=_trace
            )
            break
        except Exception:
            if attempt == 2:
                raise
    out = np.zeros((N_BAGS, NCLS), np.float32)
    for core in range(NCORE):
        o = np.asarray(res.results[core]["out"])
        for kk, bags in enumerate(meta[core]):
            if bags:
                out[np.asarray(bags)] = o[kk, 0 : len(bags)]
    kernel._last_results = res
    return out
